# revision 59
# baseline (speedup 1.0000x reference)
"""Grouped linear (MoE routing) Trainium2 kernel.

y[t] = x[t] @ weight[g_t] + bias[g_t],  g_t = group_indices[t]

Data-parallel over 8 cores (8192 tokens each), weights replicated.

Routing metadata (counting sort of tokens by group, slot capacities per
group padded to 128) is computed on the HOST from group_indices and passed
to the device as two small index tensors:
  ix [128, nslots/16] int16 — dma_gather indices in wrap-16 layout
                              (pad slots point at row 0),
  yo [128, ntiles]    int32 — output scatter row offsets per tile
                              (pad slots hold an OOB sentinel).

Device loop per core:
  1. dma_gather(transpose=True) fetches x rows in sorted order directly as
     contraction-major tiles -> lhsT without any on-chip transpose.
  2. Grouped GEMM: per 128-token tile, 8 K-chunks x 2 N-chunks of
     (K=128, M=128, N=512) bf16 matmuls accumulate f32 in PSUM; group
     weights stream through SBUF double-buffered. Gathers are interleaved
     with the tile loop so the GpSimd (SWDGE) FIFO never head-of-line
     blocks the per-tile output scatters.
  3. DVE fuses bias add (pre-broadcast per group) with PSUM->SBUF copy;
     indirect_dma_start scatters f32 rows to out[token], skipping pads
     via bounds_check.
"""

import sys

import numpy as np

sys.path.insert(0, "/opt/trn_rl_repo")

from concourse import bacc, bass, mybir, tile  # noqa: E402

N_CORES = 8
BATCH = 65536
TOK = BATCH // N_CORES  # tokens per core
DIN = 1024
DOUT = 1024
NG = 8
P = 128

FP32 = mybir.dt.float32
BF16 = mybir.dt.bfloat16
I32 = mybir.dt.int32
I16 = mybir.dt.int16

SENTINEL = 9999  # > TOK-1: skipped by bounds_check on output scatter
GCH = 512  # slots per gather chunk (1024 idxs overflows the
# single-packet SWDGE gather: 64 descs/lane kills the exec unit)

Alu = mybir.AluOpType


def build_kernel(cap):
    """cap[g] = static slot capacity of group g (multiple of 128, >=
    per-core count of group g on every core)."""
    cap = [int(c) for c in cap]
    assert all(c % P == 0 for c in cap) and sum(cap) % P == 0
    nslots = sum(cap)
    ntiles = nslots // P
    n_chunks = (nslots + GCH - 1) // GCH

    tile_group = []
    for g in range(NG):
        tile_group += [g] * (cap[g] // P)
    # runs of consecutive tiles sharing a group: [(g, first_tile), ...]
    runs = []
    for t, g in enumerate(tile_group):
        if not runs or runs[-1][0] != g:
            runs.append((g, t))
    run_of_tile = {}
    for r, (_, t0) in enumerate(runs):
        run_of_tile[t0] = r

    nc = bacc.Bacc(
        "TRN2",
        target_bir_lowering=False,
        debug=False,
        num_devices=N_CORES,
        num_swdge_queues=2,  # gathers on q1, output scatters on q0 so the
        # SDMA engines round-robin between them at packet granularity
        dynamic_dma_scratch_size=32768,  # 2048-desc SWDGE FIFO per queue:
        # 8 output scatters in flight instead of 4
    )

    x_d = nc.dram_tensor("x", [TOK, DIN], BF16, kind="ExternalInput").ap()
    w_d = nc.dram_tensor("w", [NG, DIN, DOUT], BF16, kind="ExternalInput").ap()
    # bias pre-broadcast to all 128 partitions and pre-cast to f32 on host
    b_d = nc.dram_tensor("b", [P, NG, DOUT], FP32, kind="ExternalInput").ap()
    ix_d = nc.dram_tensor("ix", [P, nslots // 16], I16, kind="ExternalInput").ap()
    yo_d = nc.dram_tensor("yo", [P, ntiles], I32, kind="ExternalInput").ap()
    # Tile serializes indirect writes per DRAM tensor (conservative WAW).
    # Alternating tiles between two outputs doubles scatter concurrency;
    # the host sums them (unwritten rows stay zero).
    out_ds = [
        nc.dram_tensor(f"out{k}", [TOK, DOUT], FP32, kind="ExternalOutput").ap()
        for k in range(6)
    ]

    with tile.TileContext(nc) as tc:
        with (
            tc.tile_pool(name="meta", bufs=1) as meta,
            tc.tile_pool(name="wpool", bufs=2) as wpool,
            tc.tile_pool(name="gpool", bufs=6) as gpool,
            tc.tile_pool(name="ypool", bufs=8) as ypool,
            tc.tile_pool(name="psum", bufs=8, space="PSUM") as psum,
        ):
            idx16 = meta.tile([P, nslots // 16], I16, tag="idx16")
            nc.sync.dma_start(out=idx16[:], in_=ix_d[:])
            yoff = meta.tile([P, ntiles], I32, tag="yoff")
            nc.sync.dma_start(out=yoff[:], in_=yo_d[:])

            w_sb = {}

            def load_w(g):
                wt = wpool.tile([P, DIN // P, DOUT], BF16, tag="w")
                nc.sync.dma_start(
                    out=wt[:], in_=w_d[g].rearrange("(c p) j -> p c j", p=P)
                )
                w_sb[g] = wt

            # chunk table: first 512 slots split 4x128 so tile 0's gather
            # lands ~10us earlier (the ext-isa lib load already costs ~17us
            # before any gather can execute); then uniform 512s
            chunks = [(0, P), (P, P), (2 * P, P), (3 * P, P)]
            s = 4 * P
            while s < nslots:
                n = min(GCH, nslots - s)
                chunks.append((s, n))
                s += n
            n_ch = len(chunks)
            # tile t -> (chunk index, offset within chunk)
            tile_chunk = []
            for ci, (s0, n) in enumerate(chunks):
                for j in range(n // P):
                    tile_chunk.append((ci, j * P))

            gtiles = [None] * n_ch

            def issue_gather(ci):
                if ci >= n_ch or gtiles[ci] is not None:
                    return
                s0, n = chunks[ci]
                # exact-shape tile from the shared tag (slots sized to the
                # 512 max): identical SBUF layout for 128- and 512-chunks
                gt = gpool.tile([P, DIN // P, n], BF16, tag="g")
                nc.gpsimd.dma_gather(
                    gt[:],
                    x_d[:],
                    idx16[:, s0 // 16 : (s0 + n) // 16],
                    n,
                    n,
                    DIN,
                    transpose=True,
                    queue_num=1,
                    single_packet=False,
                )
                gtiles[ci] = gt

            # bias pre-broadcast on host (4 MiB): dispatch AFTER w0 on the
            # same sync queue so it can't delay the first tile's weights;
            # it is only needed once the first PSUM drain happens (~45us)
            bias_rep = meta.tile([P, NG, DOUT], FP32, tag="bias_rep")
            load_w(runs[0][0])
            nc.sync.dma_start(out=bias_rep[:], in_=b_d[:])
            if len(runs) > 1:
                load_w(runs[1][0])
            for ci in range(5):
                issue_gather(ci)

            # PE pre-warm: the first real matmul can't start before ~30us
            # (ext-isa lib load gates the first gather). Keep the PE busy on
            # zero-matmuls so the HAM clock gate is at 8/8 when real work
            # arrives instead of paying the 1.2 GHz cold ramp.
            warm_l = meta.tile([P, P], BF16, tag="warm_l")
            warm_r = meta.tile([P, 512], BF16, tag="warm_r")
            nc.vector.memset(warm_l[:], 0.0)
            nc.vector.memset(warm_r[:], 0.0)
            wps = psum.tile([P, 512], FP32, tag="acc")
            for k in range(105):
                nc.tensor.matmul(
                    out=wps[:],
                    lhsT=warm_l[:],
                    rhs=warm_r[:],
                    start=(k == 0),
                    stop=(k == 104),
                )

            LOOKAHEAD = 4
            for t in range(ntiles):
                g = tile_group[t]
                r = run_of_tile.get(t)
                if r is not None and r + 2 < len(runs):
                    load_w(runs[r + 2][0])
                ci, off = tile_chunk[t]
                for d in range(1, LOOKAHEAD + 1):
                    issue_gather(ci + d)
                gt = gtiles[ci]
                y_st = ypool.tile([P, DOUT], FP32, tag="y")
                acc0 = psum.tile([P, 512], FP32, tag="acc")
                acc1 = psum.tile([P, 512], FP32, tag="acc")
                acc = [acc0, acc1]
                for ic in range(DIN // P):
                    for jc in range(2):
                        nc.tensor.matmul(
                            out=acc[jc][:],
                            lhsT=gt[:, ic, off : off + P],
                            rhs=w_sb[g][:, ic, jc * 512 : (jc + 1) * 512],
                            start=(ic == 0),
                            stop=(ic == DIN // P - 1),
                        )
                for jc in range(2):
                    nc.vector.tensor_tensor(
                        out=y_st[:, jc * 512 : (jc + 1) * 512],
                        in0=acc[jc][:],
                        in1=bias_rep[:, g, jc * 512 : (jc + 1) * 512],
                        op=Alu.add,
                    )
                nc.gpsimd.indirect_dma_start(
                    out=out_ds[t % 6][:],
                    out_offset=bass.IndirectOffsetOnAxis(
                        ap=yoff[:, t : t + 1], axis=0
                    ),
                    in_=y_st[:],
                    in_offset=None,
                    bounds_check=TOK - 1,
                    oob_is_err=False,
                )

    nc.compile()
    return nc


def _plan_caps(gi: np.ndarray) -> np.ndarray:
    counts = np.zeros((N_CORES, NG), dtype=np.int64)
    for c in range(N_CORES):
        counts[c] = np.bincount(gi[c * TOK : (c + 1) * TOK], minlength=NG)
    mx = counts.max(axis=0)
    return ((mx + P - 1) // P) * P


def _route(gi_c: np.ndarray, cap: np.ndarray):
    """Host-side counting sort for one core: build wrap-16 gather indices
    and per-tile output scatter offsets."""
    nslots = int(cap.sum())
    gbase = np.concatenate([[0], np.cumsum(cap)[:-1]]).astype(np.int64)
    I = np.zeros(nslots, np.int32)  # pad -> row 0 (harmless gather)
    Y = np.full(nslots, SENTINEL, np.int32)  # pad -> OOB sentinel
    order = np.argsort(gi_c, kind="stable").astype(np.int32)
    sorted_gi = gi_c[order]
    starts = np.searchsorted(sorted_gi, np.arange(NG))
    ends = np.searchsorted(sorted_gi, np.arange(NG), side="right")
    for g in range(NG):
        toks = order[starts[g] : ends[g]]
        I[gbase[g] : gbase[g] + len(toks)] = toks
        Y[gbase[g] : gbase[g] + len(toks)] = toks
    idx16 = np.ascontiguousarray(
        np.tile(I.astype(np.int16).reshape(-1, 16).T, (8, 1))
    )
    yoff = np.ascontiguousarray(Y.reshape(-1, P).T).astype(np.int32)
    return idx16, yoff


LAST_RESULTS = None  # stashed BassKernelResults for external profiling


def kernel(x, weight, bias, group_indices):
    global LAST_RESULTS
    from concourse.bass_utils import run_bass_kernel_spmd

    x = np.asarray(x)
    weight = np.asarray(weight)
    bias = np.asarray(bias)
    gi = np.ascontiguousarray(np.asarray(group_indices, dtype=np.int32))

    cap = _plan_caps(gi)
    nc = build_kernel(cap)

    bias_bc = np.ascontiguousarray(
        np.broadcast_to(bias.astype(np.float32)[None], (P, NG, DOUT))
    )
    in_maps = []
    for c in range(N_CORES):
        gi_c = gi[c * TOK : (c + 1) * TOK]
        ix, yo = _route(gi_c, cap)
        in_maps.append(
            {
                "x": np.ascontiguousarray(x[c * TOK : (c + 1) * TOK]),
                "w": weight,
                "b": bias_bc,
                "ix": ix,
                "yo": yo,
            }
        )
    res = run_bass_kernel_spmd(nc, in_maps, core_ids=list(range(N_CORES)))
    LAST_RESULTS = res
    out = np.concatenate(
        [
            sum(res.results[c][f"out{k}"] for k in range(1, 6)) + res.results[c]["out0"]
            for c in range(N_CORES)
        ],
        axis=0,
    )
    return out
